# revision 1
# baseline (speedup 1.0000x reference)
"""Trainium2 Bass kernel for nn_Listener (LSTM listener + dense encoders).

Reference computation (per full batch B=512):
    emb = embed_table[message]                       # [B, T, 512]
    LSTM over T=128 steps, HIDDEN=1024:
        gated = [x_t, h] @ W_cell + b_cell           # [B, 4096] (i, g, f, o)
        f = sigmoid(f + 1); c = f*c + sigmoid(i)*tanh(g); h = sigmoid(o)*tanh(c)
    images_encoded = images @ W_img + b_img          # [B, 1024]
    hidden_encoded = h @ W_hid + b_hid               # [B, 1024]
    returns (images_encoded, hidden_encoded)

Strategy (8 NeuronCores, data-parallel over batch, 64 rows/core):
  * The embedding lookup and the x-projection fold into one table:
        M2 = embed_table @ W_cell[:512] + b_cell  (+1 on the f columns)
    so the per-step x contribution is a row-gather of M2 by token id,
    injected into PSUM with a k=128 selection matmul (full 128-partition
    start=True pass that also seeds the accumulation groups).
  * Per-core batch is 64 = half the PE array's output partitions, so the
    hidden units are split in half across PSUM partition ranges:
    partitions 0:64 = (batch, units 0:512), 64:128 = (batch, units 512:1024).
    The two column-groups of the PE array run concurrently (col tiling),
    giving full 128x128 utilization for the h @ W_h recurrence.
  * Matmuls run as float32r (full-rate fp32 on the PE at N=512).
  * h is re-transposed each step with 8 PE transposes (stationary operand
    for the next step must be h^T).
"""

import os
import numpy as np

B, T = 512, 128
HIDDEN = 1024
VOCAB = 1024
EMB = 512
OUT = 1024
D_IMG = 2048
NCORES = 8
BS = B // NCORES  # 64 batch rows per core
HH = HIDDEN // 2  # 512 = per-half hidden units

_CACHE = {}


def _build_nc(n_steps: int):
    import concourse.bass as bass
    import concourse.mybir as mybir
    from concourse import bacc, tile

    f32 = mybir.dt.float32
    f32r = mybir.dt.float32r
    bf16 = mybir.dt.bfloat16
    i32 = mybir.dt.int32
    AF = mybir.ActivationFunctionType

    nc = bacc.Bacc("TRN2", target_bir_lowering=False, debug=False)

    m2p_d = nc.declare_dram_parameter("m2p", [2 * VOCAB, HH * 4], f32r, isOutput=False)
    wh_d = nc.declare_dram_parameter("wh", [HIDDEN, 4 * HIDDEN], bf16, isOutput=False)
    msg2_d = nc.declare_dram_parameter("msg2", [2 * BS, T], i32, isOutput=False)
    sfull_d = nc.declare_dram_parameter("sfull", [2 * BS, 2 * BS], f32r, isOutput=False)
    ident_d = nc.declare_dram_parameter("ident", [128, 128], f32, isOutput=False)
    imgs_d = nc.declare_dram_parameter("imgs", [128, D_IMG // 2], f32, isOutput=False)
    wimg_d = nc.declare_dram_parameter("wimg", [D_IMG, OUT], bf16, isOutput=False)
    whid_d = nc.declare_dram_parameter("whid", [HIDDEN, OUT], bf16, isOutput=False)
    o2_d = nc.declare_dram_parameter("o2", [2, 128], f32r, isOutput=False)
    bimg2_d = nc.declare_dram_parameter("bimg2", [2, OUT // 2], f32r, isOutput=False)
    bhid2_d = nc.declare_dram_parameter("bhid2", [2, OUT // 2], f32r, isOutput=False)
    oimg_d = nc.declare_dram_parameter("oimg", [128, OUT // 2], f32, isOutput=True)
    ohid_d = nc.declare_dram_parameter("ohid", [128, OUT // 2], f32, isOutput=True)

    def r(ap):  # matmul operands are float32r-typed tiles already
        return ap

    def pe_gate(dep_insts, reason="unused"):
        return None

    def after(mm, gate):
        return mm

    with tile.TileContext(nc) as tc:
        with (
            tc.tile_pool(name="wpool", bufs=1) as wpool,
            tc.tile_pool(name="const", bufs=1) as cpool,
            tc.tile_pool(name="xg", bufs=2) as xgpool,
            tc.tile_pool(name="state", bufs=2) as stpool,
            tc.tile_pool(name="act", bufs=1) as apool,
            tc.tile_pool(name="wstream", bufs=2) as wspool,
            tc.tile_pool(name="outs", bufs=1) as opool,
            tc.tile_pool(name="psum", bufs=1, space="PSUM") as pspool,
        ):
            # ---- constants / small inputs ----
            msg2 = cpool.tile([2 * BS, T], i32, tag="msg2")
            nc.sync.dma_start(msg2[:], msg2_d[:])
            sfull = cpool.tile([2 * BS, 2 * BS], f32r, tag="sfull")
            sfull_dma = nc.sync.dma_start(sfull[:], sfull_d[:])
            ident = cpool.tile([128, 128], f32, tag="ident")
            ident_dma = nc.sync.dma_start(ident[:], ident_d[:])
            o2 = cpool.tile([2, 128], f32r, tag="o2")
            o2_dma = nc.sync.dma_start(o2[:], o2_d[:])
            bimg2 = cpool.tile([2, OUT // 2], f32r, tag="bimg2")
            bimg2_dma = nc.sync.dma_start(bimg2[:], bimg2_d[:])
            bhid2 = cpool.tile([2, OUT // 2], f32r, tag="bhid2")
            bhid2_dma = nc.sync.dma_start(bhid2[:], bhid2_d[:])

            # ---- W_h resident in SBUF: 8 chunks of [128, 4096] ----
            wh_sb = []
            wh_dmas = []
            for ci in range(8):
                wt = wpool.tile([128, 4 * HIDDEN], bf16, tag=f"wh{ci}")
                wh_dmas.append(nc.sync.dma_start(wt[:], wh_d[128 * ci : 128 * (ci + 1), :]))
                wh_sb.append(wt)

            # ---- images transposed up front (also PE warmup) ----
            # imgs packed [128, 1024]: partitions 0:64 = batch x feats 0:1024,
            # partitions 64:128 = batch x feats 1024:2048. One full 128x128
            # transpose then yields two stationary chunks at once.
            imgs = cpool.tile([128, D_IMG // 2], f32, tag="imgs")
            nc.sync.dma_start(imgs[:], imgs_d[:])
            imT = cpool.tile([128, D_IMG // 2], bf16, tag="imT")
            for half in range(2):
                tp = pspool.tile([128, 8 * BS], f32, tag="tp")
                for q in range(4):
                    qq = 4 * half + q
                    nc.tensor.transpose(
                        out=tp[:, 128 * q : 128 * (q + 1)],
                        in_=imgs[:, 128 * qq : 128 * (qq + 1)],
                        identity=ident[:],
                    )
                nc.vector.tensor_copy(imT[:, 512 * half : 512 * (half + 1)], tp[:])

            # one gate for all the startup DMAs the PE will touch
            g_start = pe_gate([sfull_dma, o2_dma, bimg2_dma, bhid2_dma] + wh_dmas)

            # ---- LSTM state init ----
            c_prev = stpool.tile([128, HH], f32, tag="c")
            nc.vector.memset(c_prev[:], 0.0)
            hT_cur = None

            gate_sl = [slice(HH * b, HH * (b + 1)) for b in range(4)]

            def hT_sl(hT, ci):
                # packed-transpose layout: pair q holds chunk q (cols 0:64)
                # and chunk q+4 (cols 64:128) at col block 128*q
                q, hi = (ci - 4, 64) if ci >= 4 else (ci, 0)
                return hT[:, 128 * q + hi : 128 * q + hi + 64]

            def imT_sl(ci):
                q, hi = (ci - 8, 64) if ci >= 8 else (ci, 0)
                return imT[:, 128 * q + hi : 128 * q + hi + 64]

            # ---- recurrence ----
            prev_gp_w = None               # last matmul writing gp (PE)
            hT_copy = None
            for t in range(n_steps):
                xg = xgpool.tile([2 * BS, 4 * HH], f32r, tag="xg")
                gather = nc.gpsimd.indirect_dma_start(
                    out=xg[:],
                    out_offset=None,
                    in_=m2p_d[:],
                    in_offset=bass.IndirectOffsetOnAxis(ap=msg2[:, t : t + 1], axis=0),
                )

                # per-bank PSUM tiles: a gate's activation then only waits
                # for ITS bank's matmuls (tile-granular dep tracking), and a
                # consumed bank frees for the next step's X pass immediately.
                gpb = [
                    pspool.tile([128, HH], f32, tag=f"gp{b}", name=f"gp{b}_{t}")
                    for b in range(4)
                ]
                # X pass: seeds every bank (start=True over all 128 partitions)
                for b in range(4):
                    nc.tensor.matmul(
                        out=gpb[b][:],
                        lhsT=r(sfull[:]),
                        rhs=r(xg[:, gate_sl[b]]),
                        start=True,
                        stop=(t == 0),
                        skip_group_check=True,
                    )
                if t > 0:
                    # h @ W_h: bank-outer for staggered epilogue starts;
                    # A/B column groups interleaved so both strips stream.
                    for b in range(4):
                        for ci in range(8):
                            last = ci == 7
                            lhs = hT_sl(hT_cur, ci)
                            nc.tensor.matmul(
                                out=gpb[b][0:64, :],
                                lhsT=lhs,
                                rhs=r(wh_sb[ci][:, 1024 * b : 1024 * b + 512]),
                                start=False,
                                stop=last,
                                skip_group_check=True,
                            )
                            nc.tensor.matmul(
                                out=gpb[b][64:128, :],
                                lhsT=lhs,
                                rhs=r(wh_sb[ci][:, 1024 * b + 512 : 1024 * (b + 1)]),
                                start=False,
                                stop=last,
                                skip_group_check=True,
                            )

                # epilogue: gates -> c, h  (banks: 0=i, 1=g, 2=f, 3=o)
                sigi = apool.tile([128, HH], f32, tag="sigi")
                nc.scalar.activation(sigi[:], gpb[0][:], AF.Sigmoid)
                tanhg = apool.tile([128, HH], f32, tag="tanhg")
                nc.scalar.activation(tanhg[:], gpb[1][:], AF.Tanh)
                m1 = apool.tile([128, HH], f32, tag="m1")
                nc.vector.tensor_mul(m1[:], sigi[:], tanhg[:])
                sigf = apool.tile([128, HH], f32, tag="sigf")
                nc.scalar.activation(sigf[:], gpb[2][:], AF.Sigmoid)
                cmul = apool.tile([128, HH], f32, tag="cmul")
                nc.vector.tensor_mul(cmul[:], sigf[:], c_prev[:])
                c_new = stpool.tile([128, HH], f32, tag="c")
                nc.vector.tensor_add(c_new[:], cmul[:], m1[:])
                # sig_o issued BEFORE tanh_c: ACT is strict FIFO, and sig_o is
                # the last reader of the gates PSUM -- issuing it early
                # releases the banks for the next step's matmuls.
                sigo = apool.tile([128, HH], f32, tag="sigo")
                nc.scalar.activation(sigo[:], gpb[3][:], AF.Sigmoid)
                tanhc = apool.tile([128, HH], f32, tag="tanhc")
                nc.scalar.activation(tanhc[:], c_new[:], AF.Tanh)
                h = apool.tile([128, HH], f32, tag="h")
                h_op = nc.vector.tensor_mul(h[:], sigo[:], tanhc[:])

                # h -> h^T: 4 full [128,128] PE transposes (each yields the
                # stationary slices for chunks q and q+4)
                tp = pspool.tile([128, 8 * BS], f32, tag="tp")
                for q in range(4):
                    nc.tensor.transpose(
                        out=tp[:, 128 * q : 128 * (q + 1)],
                        in_=h[:, 128 * q : 128 * (q + 1)],
                        identity=ident[:],
                    )
                hT_next = stpool.tile([128, 8 * BS], bf16, tag="hT")
                hT_copy = nc.vector.tensor_copy(hT_next[:], tp[:])

                c_prev = c_new
                hT_cur = hT_next

            # ---- hidden encoder: out = h @ W_hid + b_hid ----
            ohp = pspool.tile([128, OUT // 2], f32, tag="ohp")
            g_bh = pe_gate([g_start, hT_copy])
            mm = nc.tensor.matmul(
                out=ohp[:], lhsT=r(o2[:]), rhs=r(bhid2[:]),
                start=True, stop=False, skip_group_check=True,
            )
            after(mm, g_bh)
            for ci in range(8):
                wt = wspool.tile([128, OUT], bf16, tag="whid")
                wdma = nc.sync.dma_start(wt[:], whid_d[128 * ci : 128 * (ci + 1), :])
                g_w = pe_gate([wdma])
                last = ci == 7
                lhs = hT_sl(hT_cur, ci)
                mm = nc.tensor.matmul(
                    out=ohp[0:64, :], lhsT=lhs, rhs=r(wt[:, 0:512]),
                    start=False, stop=last, skip_group_check=True,
                )
                after(mm, g_w)
                mm = nc.tensor.matmul(
                    out=ohp[64:128, :], lhsT=lhs, rhs=r(wt[:, 512:1024]),
                    start=False, stop=last, skip_group_check=True,
                )
                after(mm, g_w)
            ohid_sb = opool.tile([128, OUT // 2], f32, tag="ohid")
            nc.vector.tensor_copy(ohid_sb[:], ohp[:])
            nc.sync.dma_start(ohid_d[:], ohid_sb[:])

            # ---- images encoder: out = images @ W_img + b_img ----
            oip = pspool.tile([128, OUT // 2], f32, tag="oip")
            mm = nc.tensor.matmul(
                out=oip[:], lhsT=r(o2[:]), rhs=r(bimg2[:]),
                start=True, stop=False, skip_group_check=True,
            )
            after(mm, g_bh)
            for ci in range(16):
                wt = wspool.tile([128, OUT], bf16, tag="wimg")
                wdma = nc.sync.dma_start(wt[:], wimg_d[128 * ci : 128 * (ci + 1), :])
                g_w = pe_gate([wdma])
                last = ci == 15
                lhs = imT_sl(ci)
                mm = nc.tensor.matmul(
                    out=oip[0:64, :], lhsT=lhs, rhs=r(wt[:, 0:512]),
                    start=False, stop=last, skip_group_check=True,
                )
                after(mm, g_w)
                mm = nc.tensor.matmul(
                    out=oip[64:128, :], lhsT=lhs, rhs=r(wt[:, 512:1024]),
                    start=False, stop=last, skip_group_check=True,
                )
                after(mm, g_w)
            oimg_sb = opool.tile([128, OUT // 2], f32, tag="oimg")
            nc.vector.tensor_copy(oimg_sb[:], oip[:])
            nc.sync.dma_start(oimg_d[:], oimg_sb[:])

    nc.compile()
    return nc


def _host_prep(images, embed_table, W_cell, b_cell, W_img, b_img, W_hid, b_hid,
               message):
    """Builds the per-core input maps (all host-side preprocessing)."""
    from ml_dtypes import bfloat16

    W_x = W_cell[:EMB]          # [512, 4096]
    W_h = np.ascontiguousarray(W_cell[EMB:]).astype(bfloat16)  # [1024, 4096]

    M2 = embed_table.astype(np.float32) @ W_x + b_cell  # [1024, 4096]
    M2[:, 2 * HIDDEN : 3 * HIDDEN] += 1.0  # fold the f-gate +1.0
    # row 2v+h = [i_h, g_h, f_h, o_h] halves of vocab row v
    M2p = np.ascontiguousarray(
        M2.reshape(VOCAB, 4, 2, HH).transpose(0, 2, 1, 3).reshape(2 * VOCAB, 4 * HH)
    )

    sfull = np.zeros((2 * BS, 2 * BS), np.float32)
    for m in range(BS):
        sfull[2 * m, m] = 1.0
        sfull[2 * m + 1, BS + m] = 1.0

    ident = np.eye(128, dtype=np.float32)

    o2 = np.zeros((2, 128), np.float32)
    o2[0, 0:64] = 1.0
    o2[1, 64:128] = 1.0

    W_img_b = W_img.astype(bfloat16)
    W_hid_b = W_hid.astype(bfloat16)
    bimg2 = np.stack([b_img[: OUT // 2], b_img[OUT // 2 :]]).astype(np.float32)
    bhid2 = np.stack([b_hid[: OUT // 2], b_hid[OUT // 2 :]]).astype(np.float32)

    in_maps = []
    for core in range(NCORES):
        sl = slice(core * BS, (core + 1) * BS)
        msg = message[sl]  # [64, T] int32
        msg2 = np.empty((2 * BS, T), np.int32)
        msg2[0::2] = 2 * msg
        msg2[1::2] = 2 * msg + 1
        in_maps.append(
            {
                "m2p": M2p,
                "wh": W_h,
                "msg2": msg2,
                "sfull": sfull,
                "ident": ident,
                "imgs": np.concatenate(
                    [images[sl, : D_IMG // 2], images[sl, D_IMG // 2 :]], axis=0
                ),
                "wimg": W_img_b,
                "whid": W_hid_b,
                "o2": o2,
                "bimg2": bimg2,
                "bhid2": bhid2,
            }
        )
    return in_maps


def kernel(images, embed_table, W_cell, b_cell, W_img, b_img, W_hid, b_hid,
           message):
    import sys
    if "/opt/trn_rl_repo" not in sys.path:
        sys.path.insert(0, "/opt/trn_rl_repo")
    from concourse.bass_utils import run_bass_kernel_spmd

    images = np.asarray(images, np.float32)
    embed_table = np.asarray(embed_table, np.float32)
    W_cell = np.asarray(W_cell, np.float32)
    b_cell = np.asarray(b_cell, np.float32)
    W_img = np.asarray(W_img, np.float32)
    b_img = np.asarray(b_img, np.float32)
    W_hid = np.asarray(W_hid, np.float32)
    b_hid = np.asarray(b_hid, np.float32)
    message = np.asarray(message, np.int32)

    n_steps = T
    if "nc" not in _CACHE or _CACHE.get("n_steps") != n_steps:
        _CACHE["nc"] = _build_nc(n_steps)
        _CACHE["n_steps"] = n_steps
    nc = _CACHE["nc"]

    in_maps = _host_prep(
        images, embed_table, W_cell, b_cell, W_img, b_img, W_hid, b_hid, message
    )
    res = run_bass_kernel_spmd(nc, in_maps, core_ids=list(range(NCORES)))
    results = res.results

    images_encoded = np.empty((B, OUT), np.float32)
    hidden_encoded = np.empty((B, OUT), np.float32)
    for core in range(NCORES):
        sl = slice(core * BS, (core + 1) * BS)
        oi = results[core]["oimg"]
        oh = results[core]["ohid"]
        images_encoded[sl, : OUT // 2] = oi[0:64]
        images_encoded[sl, OUT // 2 :] = oi[64:128]
        hidden_encoded[sl, : OUT // 2] = oh[0:64]
        hidden_encoded[sl, OUT // 2 :] = oh[64:128]
    return images_encoded, hidden_encoded

